# revision 37
# baseline (speedup 1.0000x reference)
"""Trainium2 Bass kernel for nn_BridgeModule (vision->text cross-attention + FFN).

Data-parallel over batch (B=8, one batch element per NeuronCore), channel-major
dataflow (features on partitions, tokens on the free dim).

v2 design:
  - fp8e4(+DoubleRow) attention path: vision proj, K/V, Q, scores, ctx, O all
    run in fp8 (attention output is ~3.5% of |x|, so fp8 error is negligible
    in the final output); FFN stays bf16.
  - LayerNorms are folded into the consumer weights host-side:
      ln_w into wq/f1 rows; ln_b into wq_b/f1_b (exact).  Device computes only
      per-token mean/rstd rows and applies them during PSUM eviction
      (q = r*qraw + (-m*r)*colsum(wq') [+qb]); nt/nx are never materialized.
  - Residual is applied on device; single fp32 output.
  - fp8 tensors are scaled (weights x32, normalized ctx x16) to stay in
    e4m3's normal range; descales fold into PSUM-eviction scale slots.
  - All heads' softmax reciprocals are bounced through DRAM in one batch per
    1024-token block (partition-broadcast), not per head.
"""

import numpy as np
import ml_dtypes

import concourse.bass as bass
import concourse.tile as tile
import concourse.mybir as mybir
from concourse import bacc
from concourse.bass_utils import run_bass_kernel_spmd

# ---------------------------------------------------------------- constants
B, SV, SQ = 8, 257, 2048
DV, DM, H = 1024, 2304, 8
DK = DM // H            # 288
DKP = 384               # padded head dim (3 x 128)
DQP = H * DKP           # 3072
DF = 4 * DM             # 9216
SVP = 384               # padded vision tokens
EPS = 1e-5
P = 128
SCALE = 1.0 / float(np.sqrt(np.float32(DK)))

KO_DM = DM // P         # 18
KO_QP = DQP // P        # 24
KO_DV = DV // P         # 8
KO_DF = DF // P         # 72
HC = DKP // P           # 3 contraction chunks per head
ST = SVP // P           # 3 vision-token partition tiles
NB = 4                  # attention token blocks
NBS = SQ // NB          # 512
NT = SQ // 512          # matmul free-dim tiles of 512

WS = 32.0               # fp8 weight scale
CR = 16.0               # fp8 normalized-ctx scale

BF = mybir.dt.bfloat16
F32 = mybir.dt.float32
F8 = mybir.dt.float8e4
bf16 = ml_dtypes.bfloat16
f8e4 = ml_dtypes.float8_e4m3
DR = mybir.MatmulPerfMode.DoubleRow

AF = mybir.ActivationFunctionType
OP = mybir.AluOpType

_NC_CACHE = {}


def _dq(nc, i):
    """Alternate bulk DMAs between the two HW DGE queues (SP / ACT)."""
    return nc.sync if i % 2 == 0 else nc.scalar


def _pbcast(ap2d, p=P):
    """[1, ...] AP -> [p, ...] AP with partition stride 0 (for DMA broadcast)."""
    aplist = [list(x) for x in ap2d.ap]
    return bass.AP(tensor=ap2d.tensor, offset=ap2d.offset,
                   ap=[[0, p]] + aplist[1:])


def _build_nc(has_qb, has_wob):
    nc = bacc.Bacc(target_bir_lowering=False)
    with tile.TileContext(nc) as tc:
        _emit(nc, tc, has_qb, has_wob)
    nc.compile()
    return nc


def _emit(nc, tc, has_qb, has_wob):
    import os
    kph = int(os.environ.get("KPH", "9"))
    with tc.tile_pool(name="dram", bufs=1, space="DRAM") as dram:
        # ---------------- external I/O (SBUF-image layouts, host-prepped)
        def ein(name, shape, dtype):
            return dram.tile(list(shape), dtype, kind="ExternalInput",
                             name=name, uniquify=False)

        te = ein("te", [P, KO_DM, SQ], BF)
        te8 = ein("te8", [P, KO_DM, SQ], F8)
        vf8 = ein("vf8", [P, KO_DV, SVP], F8)
        vp8 = ein("vp8", [KO_DM, P, KO_DV, P], F8)
        wq8 = ein("wq8", [KO_QP, P, KO_DM, P], F8)
        wk8 = ein("wk8", [KO_QP, P, KO_DM, P], F8)
        wv8 = ein("wv8", [DQP // 512, P, KO_DM, 512], F8)
        wo8 = ein("wo8", [KO_DM, P, KO_QP, P], F8)
        f1t = ein("f1t", [KO_DF, P, KO_DM, P], BF)
        f2t = ein("f2t", [KO_DM, P, KO_DF, P], BF)
        vp_bt = ein("vp_bt", [P, KO_DM], F32)
        wkb_t = ein("wkb_t", [P, KO_QP], F32)
        qs_c = ein("qs_c", [P, KO_QP], F32)
        qb_c = ein("qb_c", [P, KO_QP], F32)
        wvb = ein("wvb", [1, DQP], F32)
        wob_t = ein("wob_t", [P, KO_DM], F32)
        f1b_t = ein("f1b_t", [P, KO_DF], F32)
        f1s_c = ein("f1s_c", [P, KO_DF], F32)
        f2b_t = ein("f2b_t", [P, KO_DM], F32)
        out = dram.tile([P, KO_DM, SQ], F32, kind="ExternalOutput",
                        name="out", uniquify=False)

        # DRAM scratch
        x_dram = dram.tile([P, KO_DM, SQ], BF, name="x_dram")
        rec_dram = dram.tile([1, NB * H * NBS], BF, name="rec_dram")

        with tc.tile_pool(name="consts", bufs=1) as consts, \
             tc.tile_pool(name="psum", bufs=4, space="PSUM") as psum, \
             tc.tile_pool(name="psums", bufs=4, space="PSUM") as psums:

            ones_bf = consts.tile([P, 1], BF)
            nc.vector.memset(ones_bf[:], 1.0)
            ones_f8 = consts.tile([P, 1], F8)
            nc.vector.memset(ones_f8[:], 1.0)

            def cload(src, shape):
                t = consts.tile(list(shape), F32, tag=f"c_{src.name}")
                nc.sync.dma_start(t[:], src[:])
                return t

            vp_b = cload(vp_bt, [P, KO_DM])
            wk_b = cload(wkb_t, [P, KO_QP])
            qs_v = cload(qs_c, [P, KO_QP])
            qb_v = cload(qb_c, [P, KO_QP]) if has_qb else None
            wo_b = cload(wob_t, [P, KO_DM]) if has_wob else None
            f1_b = cload(f1b_t, [P, KO_DF])
            f1s_v = cload(f1s_c, [P, KO_DF])
            f2_b = cload(f2b_t, [P, KO_DM])

            with tc.tile_pool(name="fpre", bufs=1) as fpre:
              with tc.tile_pool(name="kvpool", bufs=1) as kvpool:
                kcm = kvpool.tile([P, KO_QP, SVP], F8)    # keys, channel-major
                v_tm = kvpool.tile([P, ST, DQP], F8)      # values, token-major
                # te streams on the sync queue from t=0 (LN1 stats are the
                # critical path for Q); vision weights ride the scalar queue
                # concurrently.
                with tc.tile_pool(name="l1pool", bufs=1) as l1pool:
                    te_sb = l1pool.tile([P, KO_DM, SQ], BF, tag="ln1_src")
                    if kph >= 2:
                        for m in range(KO_DM):
                            nc.sync.dma_start(te_sb[:, m], te[:, m])
                    if kph >= 1:
                        _vision_kv(nc, tc, psum, vf8, vp8, wk8, wv8,
                                   vp_b, wk_b, wvb, kcm, v_tm)
                    if kph >= 2:
                        rq_d, mr_d = _ln_stats_sb(
                            nc, tc, psums, l1pool, ones_bf, te_sb, dram,
                            "ln1", 1.0 / WS)
                with tc.tile_pool(name="l1bc", bufs=1) as l1bc:
                    if kph >= 3:
                        rq_b = l1bc.tile([P, SQ], BF, tag="rq_b")
                        nc.gpsimd.dma_start(rq_b[:], _pbcast(rq_d[:]))
                        mr_b = l1bc.tile([P, SQ], BF, tag="mr_b")
                        nc.gpsimd.dma_start(mr_b[:], _pbcast(mr_d[:]))
                    if kph >= 4:
                        _q_attention(nc, tc, psum, psums, ones_bf,
                                     ones_f8, kcm, v_tm, te8, wq8, rq_b,
                                     mr_b, qs_v, qb_v, wo8, wo_b, te,
                                     x_dram, rec_dram, dram, fpre, f1t)

              if kph >= 6:
                  _ffn(nc, tc, psum, f1t, f2t, x_dram,
                       _emit.r2_d, _emit.mr2_d, f1s_v, f1_b,
                       f2_b, out, _emit.ffn_pre)


def _vision_kv(nc, tc, psum, vf8, vp8, wk8, wv8, vp_b, wk_b, wvb, kcm, v_tm):
    """pv = vp.T @ vf + vp_b (fp8); keys kcm = wk.T @ pv + wk_b (channel-major
    fp8); values v_tm = pv.T @ wv + wv_b (token-major fp8). All DoubleRow."""
    with tc.tile_pool(name="vision", bufs=1) as vision, \
         tc.tile_pool(name="vwork", bufs=3) as vwork:
        vf_sb = vision.tile([P, KO_DV, SVP], F8)
        nc.scalar.dma_start(vf_sb[:], vf8[:])
        wv_bb = vision.tile([P, DQP], F32)
        nc.gpsimd.dma_start(wv_bb[:], _pbcast(wvb[:]))
        pv = vision.tile([P, KO_DM, SVP], F8)
        for m in range(KO_DM):
            w_sl = vwork.tile([P, KO_DV, P], F8, tag="vp_sl")
            nc.scalar.dma_start(w_sl[:], vp8[m])
            ps = psum.tile([P, 512], F32, tag="ps_a")
            for k2 in range(KO_DV // 2):
                nc.tensor.matmul(ps[:, :SVP], w_sl[:, 2 * k2:2 * k2 + 2],
                                 vf_sb[:, 2 * k2:2 * k2 + 2],
                                 start=(k2 == 0), stop=(k2 == KO_DV // 2 - 1),
                                 perf_mode=DR)
            nc.scalar.activation(pv[:, m], ps[:, :SVP], AF.Identity,
                                 bias=vp_b[:, m:m + 1], scale=1.0 / WS)

        for m in range(KO_QP):
            w_sl = vwork.tile([P, KO_DM, P], F8, tag="wk_sl")
            nc.scalar.dma_start(w_sl[:], wk8[m])
            ps = psum.tile([P, 512], F32, tag="ps_a")
            for k2 in range(KO_DM // 2):
                nc.tensor.matmul(ps[:, :SVP], w_sl[:, 2 * k2:2 * k2 + 2],
                                 pv[:, 2 * k2:2 * k2 + 2],
                                 start=(k2 == 0), stop=(k2 == KO_DM // 2 - 1),
                                 perf_mode=DR)
            nc.scalar.activation(kcm[:, m], ps[:, :SVP], AF.Identity,
                                 bias=wk_b[:, m:m + 1], scale=1.0 / WS)

        for n in range(DQP // 512):
            w_sl = vwork.tile([P, KO_DM, 512], F8, tag="wv_sl", bufs=2)
            nc.scalar.dma_start(w_sl[:], wv8[n])
            for st in range(ST):
                ps = psum.tile([P, 512], F32, tag="ps_a")
                ssl = slice(st * P, (st + 1) * P)
                for k2 in range(KO_DM // 2):
                    nc.tensor.matmul(ps[:], pv[:, 2 * k2:2 * k2 + 2, ssl],
                                     w_sl[:, 2 * k2:2 * k2 + 2],
                                     start=(k2 == 0),
                                     stop=(k2 == KO_DM // 2 - 1),
                                     perf_mode=DR)
                nc.vector.scalar_tensor_tensor(
                    v_tm[:, st, n * 512:(n + 1) * 512], ps[:], 1.0 / WS,
                    wv_bb[:, n * 512:(n + 1) * 512], OP.mult, OP.add)


def _ln_stats(nc, tc, psums, pool, ones_bf, src_dram, dram, nm, rscale):
    """Per-token mean/rstd of a [P, KO_DM, SQ] bf16 DRAM tensor (reduction over
    channels=partitions via ones-matmuls).  Returns DRAM rows (rstd*rscale,
    -mean*rstd) ready for partition-broadcast."""
    src_sb = pool.tile([P, KO_DM, SQ], BF, tag=nm + "_src")
    for m in range(KO_DM):
        _dq(nc, m).dma_start(src_sb[:, m], src_dram[:, m])
    return _ln_stats_sb(nc, tc, psums, pool, ones_bf, src_sb, dram, nm, rscale)


def _ln_stats_sb(nc, tc, psums, pool, ones_bf, src_sb, dram, nm, rscale):
    """m-outer so each te chunk's stats issue as soon as its DMA lands."""
    with tc.tile_pool(name=nm + "w", bufs=3) as work:
        sums = pool.tile([1, SQ], F32, tag=nm + "_sums")
        sumsq = pool.tile([1, SQ], F32, tag=nm + "_sumsq")
        ps2s = [psums.tile([33, 512], F32, tag="ps_st", name=f"{nm}_ps{n}")
                for n in range(NT)]
        for m in range(KO_DM):
            sq = work.tile([P, SQ], BF, tag=nm + "_sq")
            nc.vector.tensor_mul(sq[:], src_sb[:, m], src_sb[:, m])
            for n in range(NT):
                nsl = slice(n * 512, (n + 1) * 512)
                nc.tensor.matmul(ps2s[n][0:1], ones_bf[:], src_sb[:, m, nsl],
                                 start=(m == 0), stop=(m == KO_DM - 1))
                nc.tensor.matmul(ps2s[n][32:33], ones_bf[:], sq[:, nsl],
                                 start=(m == 0), stop=(m == KO_DM - 1))
        for n in range(NT):
            nsl = slice(n * 512, (n + 1) * 512)
            nc.vector.tensor_copy(sums[:, nsl], ps2s[n][0:1])
            nc.vector.tensor_copy(sumsq[:, nsl], ps2s[n][32:33])
        return _ln_finalize(nc, pool, sums, sumsq, dram, nm, rscale)


def _ln_finalize(nc, pool, sums, sumsq, dram, nm, rscale, q=None):
    """-> DRAM rows (rstd*rscale [1,SQ], -mean*rstd [1,SQ])."""
    q = q if q is not None else nc.gpsimd
    tmp = pool.tile([1, SQ], F32, tag=nm + "_fin_tmp")
    nc.vector.tensor_scalar_mul(sums[:], sums[:], 1.0 / DM)      # mean
    nc.vector.tensor_scalar_mul(sumsq[:], sumsq[:], 1.0 / DM)
    nc.vector.scalar_tensor_tensor(tmp[:], sums[:], 1.0, sums[:],
                                   OP.mult, OP.mult)             # mean^2
    nc.vector.tensor_sub(sumsq[:], sumsq[:], tmp[:])             # var
    eps_t = pool.tile([1, 1], F32, tag=nm + "_eps")
    nc.vector.memset(eps_t[:], EPS)
    nc.scalar.activation(tmp[:], sumsq[:], AF.Sqrt, bias=eps_t[:])  # std
    nc.vector.reciprocal(sumsq[:], tmp[:])                       # rstd
    nc.vector.scalar_tensor_tensor(tmp[:], sums[:], -1.0, sumsq[:],
                                   OP.mult, OP.mult)             # -mean*rstd
    if rscale != 1.0:
        nc.vector.tensor_scalar_mul(sumsq[:], sumsq[:], rscale)
    # row bounces ride the SWDGE queue so they never block the HWDGE
    # weight/activation FIFOs
    r_d = dram.tile([1, SQ], F32, name=nm + "_r_dram")
    q.dma_start(r_d[:], sumsq[:])
    mr_d = dram.tile([1, SQ], F32, name=nm + "_mr_dram")
    q.dma_start(mr_d[:], tmp[:])
    return r_d, mr_d


def _q_attention(nc, tc, psum, psums, ones_bf, ones_f8, kcm, v_tm, te8, wq8,
                 rq_b, mr_b, qs_v, qb_v, wo8, wo_b, te, x_dram, rec_dram,
                 dram, fpre, f1t):
    """Q projection + blocked cross-attention + O projection + residual +
    LN2 stats, fully pipelined over 512-token blocks:
      Q(0) s(0) Q(1) s(1) c/O(0) Q(2) s(2) c/O(1) ... c/O(3)
    (scores run one block ahead of ctx/O to hide the reciprocal DRAM
    bounce; q never leaves SBUF).  x -> x_dram (bf16)."""
    with tc.tile_pool(name="attn", bufs=2) as attn, \
         tc.tile_pool(name="awork", bufs=2) as awork:
        sums2 = awork.tile([1, SQ], F32, tag="x2s", bufs=1, name="x2sums")
        sumsq2 = awork.tile([1, SQ], F32, tag="x2q", bufs=1, name="x2sumsq")

        def qproj(nb):
            nsl = slice(nb * NBS, (nb + 1) * NBS)
            te8b = attn.tile([P, KO_DM, NBS], F8, tag="te8b", bufs=1,
                             name=f"te8b{nb}")
            _dq(nc, nb).dma_start(te8b[:], te8[:, :, nsl])
            q_blk = attn.tile([P, KO_QP, NBS], F8, tag="q_blk",
                              name=f"q_blk{nb}")
            for m in range(KO_QP):
                w_sl = awork.tile([P, KO_DM, P], F8, tag="wq_sl", bufs=3)
                _dq(nc, m).dma_start(w_sl[:], wq8[m])
                ps = psum.tile([P, NBS], F32, tag="ps_a",
                               name=f"ps_q{nb}_{m}")
                for k2 in range(KO_DM // 2):
                    nc.tensor.matmul(ps[:], w_sl[:, 2 * k2:2 * k2 + 2],
                                     te8b[:, 2 * k2:2 * k2 + 2],
                                     start=(k2 == 0),
                                     stop=(k2 == KO_DM // 2 - 1),
                                     perf_mode=DR)
                tmp = awork.tile([P, NBS], F32, tag="q_tmp", bufs=3)
                nc.vector.tensor_mul(tmp[:], ps[:], rq_b[:, nsl])
                if qb_v is None:
                    nc.vector.scalar_tensor_tensor(
                        q_blk[:, m], mr_b[:, nsl], qs_v[:, m:m + 1],
                        tmp[:], OP.mult, OP.add)
                else:
                    nc.vector.scalar_tensor_tensor(
                        tmp[:], mr_b[:, nsl], qs_v[:, m:m + 1],
                        tmp[:], OP.mult, OP.add)
                    nc.vector.tensor_scalar_add(q_blk[:, m], tmp[:],
                                                qb_v[:, m:m + 1])
            return q_blk

        def scores(nb, q_blk):
            expT = attn.tile([P, H * ST, NBS], F8, tag="expT",
                             name=f"expT{nb}")
            rec = awork.tile([1, H * NBS], BF, tag="rec", bufs=1,
                             name=f"rec{nb}")
            for h in range(H):
                nc.vector.memset(expT[:, HC * h + HC - 1], 0.0)
                ps_sum = psums.tile([33, NBS], F32, tag="ps_st",
                                    name=f"ps_sum{nb}_{h}")
                for st in range(ST):
                    ps_s = psum.tile([P, NBS], F32, tag="ps_a",
                                     name=f"ps_s{nb}_{h}_{st}")
                    ssl = slice(st * P, (st + 1) * P)
                    nc.tensor.matmul(ps_s[:], kcm[:, HC * h:HC * h + 2, ssl],
                                     q_blk[:, HC * h:HC * h + 2],
                                     start=True, stop=False, perf_mode=DR)
                    nc.tensor.matmul(ps_s[:], kcm[:, HC * h + 2, ssl],
                                     q_blk[:, HC * h + 2],
                                     start=False, stop=True)
                    if st < ST - 1:
                        nc.scalar.activation(expT[:, HC * h + st], ps_s[:],
                                             AF.Exp, scale=SCALE)
                    else:
                        nc.scalar.activation(expT[0:1, HC * h + st],
                                             ps_s[0:1], AF.Exp, scale=SCALE)
                    nc.tensor.matmul(ps_sum[0:1], ones_f8[:],
                                     expT[:, HC * h + st],
                                     start=(st == 0), stop=(st == ST - 1))
                nc.vector.tensor_scalar_mul(ps_sum[0:1], ps_sum[0:1],
                                            1.0 / CR)
                with nc.allow_low_precision(
                        reason="softmax recip in bf16 is plenty"):
                    nc.vector.reciprocal(rec[:, h * NBS:(h + 1) * NBS],
                                         ps_sum[0:1])
            roff = nb * H * NBS
            nc.gpsimd.dma_start(rec_dram[:, roff:roff + H * NBS], rec[:])
            rec_b = attn.tile([P, H * NBS], BF, tag="rec_b", bufs=1,
                              name=f"rec_b{nb}")
            nc.gpsimd.dma_start(rec_b[:],
                                _pbcast(rec_dram[:, roff:roff + H * NBS]))
            return expT, rec_b

        def ctx_o(nb, expT, rec_b):
            bsl = slice(nb * NBS, (nb + 1) * NBS)
            ctx8 = attn.tile([P, KO_QP, NBS], F8, tag="ctx8", bufs=1,
                             name=f"ctx8_{nb}")
            for h in range(H):
                for d3 in range(HC):
                    dsl = slice((HC * h + d3) * P, (HC * h + d3 + 1) * P)
                    ps_c = psum.tile([P, NBS], F32, tag="ps_a",
                                     name=f"ps_c{nb}_{h}_{d3}")
                    nc.tensor.matmul(ps_c[:], v_tm[:, 0:2, dsl],
                                     expT[:, HC * h:HC * h + 2],
                                     start=True, stop=False, perf_mode=DR)
                    nc.tensor.matmul(ps_c[:], v_tm[:, 2, dsl],
                                     expT[:, HC * h + 2],
                                     start=False, stop=True)
                    nc.vector.tensor_mul(ctx8[:, HC * h + d3], ps_c[:],
                                         rec_b[:, h * NBS:(h + 1) * NBS])

            # O projection + residual -> x_dram (bf16); LN2 stats inline.
            ps_st = psums.tile([33, NBS], F32, tag="ps_st",
                               name=f"ps_st{nb}")
            for m in range(KO_DM):
                w_sl = awork.tile([P, KO_QP, P], F8, tag="wo_sl", bufs=2)
                _dq(nc, m).dma_start(w_sl[:], wo8[m])
                te_sl = awork.tile([P, NBS], BF, tag="te_res")
                _dq(nc, m + 1).dma_start(te_sl[:], te[:, m, bsl])
                x_t = awork.tile([P, NBS], BF, tag="x_t")
                sq_t = awork.tile([P, NBS], BF, tag="sq_t")
                ps = psum.tile([P, NBS], F32, tag="ps_a",
                               name=f"ps_o{nb}_{m}")
                for k2 in range(KO_QP // 2):
                    nc.tensor.matmul(ps[:], w_sl[:, 2 * k2:2 * k2 + 2],
                                     ctx8[:, 2 * k2:2 * k2 + 2],
                                     start=(k2 == 0),
                                     stop=(k2 == KO_QP // 2 - 1),
                                     perf_mode=DR)
                if wo_b is None:
                    nc.vector.scalar_tensor_tensor(
                        x_t[:], ps[:], 1.0 / (WS * CR), te_sl[:],
                        OP.mult, OP.add)
                else:
                    tmp = awork.tile([P, NBS], F32, tag="xo_tmp")
                    nc.vector.scalar_tensor_tensor(
                        tmp[:], ps[:], 1.0 / (WS * CR), wo_b[:, m:m + 1],
                        OP.mult, OP.add)
                    nc.vector.tensor_add(x_t[:], tmp[:], te_sl[:])
                nc.tensor.matmul(ps_st[0:1], ones_bf[:], x_t[:],
                                 start=(m == 0), stop=(m == KO_DM - 1))
                nc.vector.tensor_mul(sq_t[:], x_t[:], x_t[:])
                nc.tensor.matmul(ps_st[32:33], ones_bf[:], sq_t[:],
                                 start=(m == 0), stop=(m == KO_DM - 1))
                _dq(nc, m).dma_start(x_dram[:, m, bsl], x_t[:])
            nc.vector.tensor_copy(sums2[:, bsl], ps_st[0:1])
            nc.vector.tensor_copy(sumsq2[:, bsl], ps_st[32:33])

        # scores run one block ahead of ctx/O to hide the reciprocal bounce
        pend = [scores(0, qproj(0))]
        for nb in range(NB):
            if nb + 1 < NB:
                pend.append(scores(nb + 1, qproj(nb + 1)))
            ctx_o(nb, *pend[nb])
            if nb == 0:
                # FFN prefetches live in fpre (reserved OUTSIDE the attention
                # pools, so no SBUF-reuse WAR) and start as soon as block 0's
                # x rows land in DRAM -- long before attention finishes.
                x0 = fpre.tile([P, KO_DM, 512], BF, name="x_blk0")
                nc.gpsimd.dma_start(x0[:], x_dram[:, :, 0:512])
                f1p = []
                for _m in range(2):
                    w = fpre.tile([P, KO_DM, P], BF, name=f"f1pre{_m}")
                    nc.gpsimd.dma_start(w[:], f1t[_m])
                    f1p.append(w)
                _emit.ffn_pre = (x0, f1p)
        _emit.r2_d, _emit.mr2_d = _ln_finalize(
            nc, awork, sums2, sumsq2, dram, "ln2", 1.0)


def _ffn(nc, tc, psum, f1t, f2t, x_dram, r2_d, mr2_d, f1s_v, f1_b, f2_b,
         out, ffn_pre):
    """Fused FFN over 512-token phases; h never leaves SBUF.
    FFN1(n): h = gelu(r2*(f1'.T @ x) + (-m2*r2)*colsum(f1') + f1_b')
    FFN2(n): out = f2.T @ h + f2_b + x   (residual on device)
    f1 streams on the sync queue, f2 on the scalar queue; both are re-read
    per phase (~170 MB each over the whole FFN span -- well under the HBM
    budget and fully overlapped)."""
    with tc.tile_pool(name="hpool", bufs=1) as hpool, \
         tc.tile_pool(name="fwork", bufs=2) as fwork:
        x0_pre, pre = ffn_pre
        for n in range(NT):
            nsl = slice(n * 512, (n + 1) * 512)
            if n == 0:
                x_blk = x0_pre
            else:
                x_blk = fwork.tile([P, KO_DM, 512], BF, tag="x_blk",
                                   name=f"x_blk{n}")
                nc.gpsimd.dma_start(x_blk[:], x_dram[:, :, nsl])
            r2s = fwork.tile([P, 512], F32, tag="r2s", name=f"r2s{n}")
            nc.gpsimd.dma_start(r2s[:], _pbcast(r2_d[:, nsl]))
            mr2s = fwork.tile([P, 512], F32, tag="mr2s", name=f"mr2s{n}")
            nc.gpsimd.dma_start(mr2s[:], _pbcast(mr2_d[:, nsl]))
            h_sb = hpool.tile([P, KO_DF, 512], BF, tag="h_sb",
                              name=f"h_sb{n}")
            for m in range(KO_DF):
                if n == 0 and m < len(pre):
                    w_sl = pre[m]
                else:
                    w_sl = fwork.tile([P, KO_DM, P], BF, tag="f1_sl", bufs=3)
                    nc.sync.dma_start(w_sl[:], f1t[m])
                ps = psum.tile([P, 512], F32, tag="ps_a",
                               name=f"ps_f1_{n}_{m}")
                for k in range(KO_DM):
                    nc.tensor.matmul(ps[:], w_sl[:, k], x_blk[:, k],
                                     start=(k == 0), stop=(k == KO_DM - 1))
                tmp = fwork.tile([P, 512], F32, tag="h_tmp", bufs=3)
                nc.vector.tensor_mul(tmp[:], ps[:], r2s[:])
                nc.vector.scalar_tensor_tensor(tmp[:], mr2s[:],
                                               f1s_v[:, m:m + 1], tmp[:],
                                               OP.mult, OP.add)
                nc.scalar.activation(h_sb[:, m], tmp[:], AF.Gelu,
                                     bias=f1_b[:, m:m + 1])
            for m in range(KO_DM):
                w2a = fwork.tile([P, KO_DF // 2, P], BF, tag="f2_sl",
                                 bufs=3, name=f"w2a{n}_{m}")
                nc.scalar.dma_start(w2a[:], f2t[m][:, 0:KO_DF // 2])
                w2b = fwork.tile([P, KO_DF // 2, P], BF, tag="f2_sl",
                                 bufs=3, name=f"w2b{n}_{m}")
                nc.scalar.dma_start(w2b[:], f2t[m][:, KO_DF // 2:KO_DF])
                ps = psum.tile([P, 512], F32, tag="ps_a",
                               name=f"ps_f2_{n}_{m}")
                for k in range(KO_DF):
                    w2 = w2a if k < KO_DF // 2 else w2b
                    nc.tensor.matmul(ps[:], w2[:, k % (KO_DF // 2)],
                                     h_sb[:, k],
                                     start=(k == 0), stop=(k == KO_DF - 1))
                o_sl = fwork.tile([P, 512], F32, tag="o_sl")
                nc.vector.scalar_tensor_tensor(
                    o_sl[:], ps[:], f2_b[:, m:m + 1], x_blk[:, m],
                    OP.add, OP.add)
                _dq(nc, m).dma_start(out[:, m, nsl], o_sl[:])


# ------------------------------------------------------------- host wrappers

def _tile_w(w, ko, mo, dtype):
    """[K, M] weight -> [mo, 128, ko, mi] SBUF-image tiles."""
    K, M = w.shape
    mi = M // mo
    r = w.reshape(ko, P, mo, mi).transpose(2, 1, 0, 3)
    return np.ascontiguousarray(r.astype(dtype))


def _col_pad_heads(w):
    """[*, 2304] -> [*, 3072] zero-padding each head's 288 cols to 384."""
    r = np.zeros(w.shape[:-1] + (DQP,), np.float32)
    r.reshape(w.shape[:-1] + (H, DKP))[..., :DK] = \
        w.reshape(w.shape[:-1] + (H, DK))
    return r


def _row_pad_heads(w):
    """[2304, *] -> [3072, *] zero-padding each head's 288 rows to 384."""
    r = np.zeros((DQP,) + w.shape[1:], np.float32)
    r.reshape((H, DKP) + w.shape[1:])[:, :DK] = w.reshape((H, DK) + w.shape[1:])
    return r


def _vec_t(v, ko):
    """[ko*128] vector -> [128, ko] f32."""
    return np.ascontiguousarray(v.reshape(ko, P).T.astype(np.float32))


def _make_in_maps(inputs):
    inputs = {k: np.asarray(v, np.float32) for k, v in inputs.items()}

    ln1w = inputs["ln1_w"][:, None]
    ln2w = inputs["ln2_w"][:, None]
    wq_f = _col_pad_heads(inputs["wq_w"] * ln1w)        # ln1_w folded
    wk_pad = _col_pad_heads(inputs["wk_w"])
    wv_pad = _col_pad_heads(inputs["wv_w"])
    wo_pad = _row_pad_heads(inputs["wo_w"])
    f1_f = inputs["f1_w"] * ln2w                        # ln2_w folded

    qb = _col_pad_heads((inputs["wq_b"]
                         + inputs["ln1_b"] @ inputs["wq_w"])[None])[0]
    f1b = inputs["f1_b"] + inputs["ln2_b"] @ inputs["f1_w"]
    has_qb = bool(np.any(qb))
    has_wob = bool(np.any(inputs["wo_b"]))

    shared = {
        "vp8": _tile_w(inputs["vp_w"] * WS, KO_DV, KO_DM, f8e4),
        "wq8": _tile_w(wq_f * WS, KO_DM, KO_QP, f8e4),
        "wk8": _tile_w(wk_pad * WS, KO_DM, KO_QP, f8e4),
        "wv8": _tile_w(wv_pad * WS, KO_DM, DQP // 512, f8e4),
        "wo8": _tile_w(wo_pad * WS, KO_QP, KO_DM, f8e4),
        "f1t": _tile_w(f1_f, KO_DM, KO_DF, bf16),
        "f2t": _tile_w(inputs["f2_w"], KO_DF, KO_DM, bf16),
        "vp_bt": _vec_t(inputs["vp_b"], KO_DM),
        "wkb_t": _vec_t(_col_pad_heads(inputs["wk_b"][None])[0], KO_QP),
        "qs_c": _vec_t(wq_f.sum(axis=0), KO_QP),
        "qb_c": _vec_t(qb, KO_QP),
        "wvb": np.ascontiguousarray(
            _col_pad_heads(inputs["wv_b"][None]).astype(np.float32)),
        "wob_t": _vec_t(inputs["wo_b"], KO_DM),
        "f1b_t": _vec_t(f1b, KO_DF),
        "f1s_c": _vec_t(f1_f.sum(axis=0), KO_DF),
        "f2b_t": _vec_t(inputs["f2_b"], KO_DM),
    }

    text = inputs["text_embeddings"]
    vision = inputs["vision_features"]
    in_maps = []
    for b in range(B):
        te_cm = np.ascontiguousarray(
            text[b].T.reshape(KO_DM, P, SQ).transpose(1, 0, 2))
        vf_pad = np.zeros((DV, SVP), np.float32)
        vf_pad[:, :SV] = vision[b].T
        vf_b = np.ascontiguousarray(
            vf_pad.reshape(KO_DV, P, SVP).transpose(1, 0, 2).astype(f8e4))
        in_maps.append({"te": te_cm.astype(bf16), "te8": te_cm.astype(f8e4),
                        "vf8": vf_b, **shared})
    return in_maps, has_qb, has_wob


def kernel(**inputs):
    in_maps, has_qb, has_wob = _make_in_maps(inputs)

    key = ("nc", has_qb, has_wob)
    if key not in _NC_CACHE:
        _NC_CACHE[key] = _build_nc(has_qb, has_wob)
        _NC_CACHE["nc"] = _NC_CACHE[key]
    nc = _NC_CACHE[key]

    res = run_bass_kernel_spmd(nc, in_maps, core_ids=list(range(B)))

    outs = []
    for b in range(B):
        r = res.results[b]["out"]                       # [128, 18, 2048]
        outs.append(r.transpose(1, 0, 2).reshape(DM, SQ).T)
    return np.stack(outs).astype(np.float32)


if __name__ == "__main__":
    import reference
    inp = {k: np.asarray(v) for k, v in reference.setup_inputs().items()}
    got = kernel(**inp)
    exp = np.asarray(reference.reference(**inp))
    err = float(np.linalg.norm(got - exp) / np.linalg.norm(exp))
    print("Relative error:", err)


# revision 38
# speedup vs baseline: 1.0309x; 1.0309x over previous
"""Trainium2 Bass kernel for nn_BridgeModule (vision->text cross-attention + FFN).

Data-parallel over batch (B=8, one batch element per NeuronCore), channel-major
dataflow (features on partitions, tokens on the free dim).

v2 design:
  - fp8e4(+DoubleRow) attention path: vision proj, K/V, Q, scores, ctx, O all
    run in fp8 (attention output is ~3.5% of |x|, so fp8 error is negligible
    in the final output); FFN stays bf16.
  - LayerNorms are folded into the consumer weights host-side:
      ln_w into wq/f1 rows; ln_b into wq_b/f1_b (exact).  Device computes only
      per-token mean/rstd rows and applies them during PSUM eviction
      (q = r*qraw + (-m*r)*colsum(wq') [+qb]); nt/nx are never materialized.
  - Residual is applied on device; single fp32 output.
  - fp8 tensors are scaled (weights x32, normalized ctx x16) to stay in
    e4m3's normal range; descales fold into PSUM-eviction scale slots.
  - All heads' softmax reciprocals are bounced through DRAM in one batch per
    1024-token block (partition-broadcast), not per head.
"""

import numpy as np
import ml_dtypes

import concourse.bass as bass
import concourse.tile as tile
import concourse.mybir as mybir
from concourse import bacc
from concourse.bass_utils import run_bass_kernel_spmd

# ---------------------------------------------------------------- constants
B, SV, SQ = 8, 257, 2048
DV, DM, H = 1024, 2304, 8
DK = DM // H            # 288
DKP = 384               # padded head dim (3 x 128)
DQP = H * DKP           # 3072
DF = 4 * DM             # 9216
SVP = 384               # padded vision tokens
EPS = 1e-5
P = 128
SCALE = 1.0 / float(np.sqrt(np.float32(DK)))

KO_DM = DM // P         # 18
KO_QP = DQP // P        # 24
KO_DV = DV // P         # 8
KO_DF = DF // P         # 72
HC = DKP // P           # 3 contraction chunks per head
ST = SVP // P           # 3 vision-token partition tiles
NB = 4                  # attention token blocks
NBS = SQ // NB          # 512
NT = SQ // 512          # matmul free-dim tiles of 512

WS = 32.0               # fp8 weight scale
CR = 16.0               # fp8 normalized-ctx scale

BF = mybir.dt.bfloat16
F32 = mybir.dt.float32
F8 = mybir.dt.float8e4
bf16 = ml_dtypes.bfloat16
f8e4 = ml_dtypes.float8_e4m3
DR = mybir.MatmulPerfMode.DoubleRow

AF = mybir.ActivationFunctionType
OP = mybir.AluOpType

_NC_CACHE = {}


def _dq(nc, i):
    """Alternate bulk DMAs between the two HW DGE queues (SP / ACT)."""
    return nc.sync if i % 2 == 0 else nc.scalar


def _pbcast(ap2d, p=P):
    """[1, ...] AP -> [p, ...] AP with partition stride 0 (for DMA broadcast)."""
    aplist = [list(x) for x in ap2d.ap]
    return bass.AP(tensor=ap2d.tensor, offset=ap2d.offset,
                   ap=[[0, p]] + aplist[1:])


def _build_nc(has_qb, has_wob):
    nc = bacc.Bacc(target_bir_lowering=False)
    with tile.TileContext(nc) as tc:
        _emit(nc, tc, has_qb, has_wob)
    nc.compile()
    return nc


def _emit(nc, tc, has_qb, has_wob):
    import os
    kph = int(os.environ.get("KPH", "9"))
    with tc.tile_pool(name="dram", bufs=1, space="DRAM") as dram:
        # ---------------- external I/O (SBUF-image layouts, host-prepped)
        def ein(name, shape, dtype):
            return dram.tile(list(shape), dtype, kind="ExternalInput",
                             name=name, uniquify=False)

        te = ein("te", [P, KO_DM, SQ], BF)
        te8 = ein("te8", [P, KO_DM, SQ], F8)
        vf8 = ein("vf8", [P, KO_DV, SVP], F8)
        vp8 = ein("vp8", [KO_DM, P, KO_DV, P], F8)
        wq8 = ein("wq8", [KO_QP, P, KO_DM, P], F8)
        wk8 = ein("wk8", [KO_QP, P, KO_DM, P], F8)
        wv8 = ein("wv8", [DQP // 512, P, KO_DM, 512], F8)
        wo8 = ein("wo8", [KO_DM, P, KO_QP, P], F8)
        f1t = ein("f1t", [KO_DF, P, KO_DM, P], BF)
        f2t = ein("f2t", [KO_DM, P, KO_DF, P], BF)
        vp_bt = ein("vp_bt", [P, KO_DM], F32)
        wkb_t = ein("wkb_t", [P, KO_QP], F32)
        qs_c = ein("qs_c", [P, KO_QP], F32)
        qb_c = ein("qb_c", [P, KO_QP], F32)
        wvb = ein("wvb", [1, DQP], F32)
        wob_t = ein("wob_t", [P, KO_DM], F32)
        f1b_t = ein("f1b_t", [P, KO_DF], F32)
        f1s_c = ein("f1s_c", [P, KO_DF], F32)
        f2b_t = ein("f2b_t", [P, KO_DM], F32)
        out = dram.tile([P, KO_DM, SQ], F32, kind="ExternalOutput",
                        name="out", uniquify=False)

        # DRAM scratch
        x_dram = dram.tile([P, KO_DM, SQ], BF, name="x_dram")
        rec_dram = dram.tile([1, NB * H * NBS], BF, name="rec_dram")

        with tc.tile_pool(name="consts", bufs=1) as consts, \
             tc.tile_pool(name="psum", bufs=4, space="PSUM") as psum, \
             tc.tile_pool(name="psums", bufs=4, space="PSUM") as psums:

            ones_bf = consts.tile([P, 1], BF)
            nc.vector.memset(ones_bf[:], 1.0)
            ones_f8 = consts.tile([P, 1], F8)
            nc.vector.memset(ones_f8[:], 1.0 / CR)

            def cload(src, shape):
                t = consts.tile(list(shape), F32, tag=f"c_{src.name}")
                nc.sync.dma_start(t[:], src[:])
                return t

            vp_b = cload(vp_bt, [P, KO_DM])
            wk_b = cload(wkb_t, [P, KO_QP])
            qs_v = cload(qs_c, [P, KO_QP])
            qb_v = cload(qb_c, [P, KO_QP]) if has_qb else None
            wo_b = cload(wob_t, [P, KO_DM]) if has_wob else None
            f1_b = cload(f1b_t, [P, KO_DF])
            f1s_v = cload(f1s_c, [P, KO_DF])
            f2_b = cload(f2b_t, [P, KO_DM])

            with tc.tile_pool(name="fpre", bufs=1) as fpre:
              with tc.tile_pool(name="kvpool", bufs=1) as kvpool:
                kcm = kvpool.tile([P, KO_QP, SVP], F8)    # keys, channel-major
                v_tm = kvpool.tile([P, ST, DQP], F8)      # values, token-major
                # te streams on the sync queue from t=0 (LN1 stats are the
                # critical path for Q); vision weights ride the scalar queue
                # concurrently.
                with tc.tile_pool(name="l1pool", bufs=1) as l1pool:
                    te_sb = l1pool.tile([P, KO_DM, SQ], BF, tag="ln1_src")
                    if kph >= 2:
                        for m in range(KO_DM):
                            nc.sync.dma_start(te_sb[:, m], te[:, m])
                    if kph >= 1:
                        _vision_kv(nc, tc, psum, vf8, vp8, wk8, wv8,
                                   vp_b, wk_b, wvb, kcm, v_tm)
                    if kph >= 2:
                        rq_d, mr_d = _ln_stats_sb(
                            nc, tc, psums, l1pool, ones_bf, te_sb, dram,
                            "ln1", 1.0 / WS)
                with tc.tile_pool(name="l1bc", bufs=1) as l1bc:
                    if kph >= 3:
                        rq_b = l1bc.tile([P, SQ], BF, tag="rq_b")
                        nc.gpsimd.dma_start(rq_b[:], _pbcast(rq_d[:]))
                        mr_b = l1bc.tile([P, SQ], BF, tag="mr_b")
                        nc.gpsimd.dma_start(mr_b[:], _pbcast(mr_d[:]))
                    if kph >= 4:
                        _q_attention(nc, tc, psum, psums, ones_bf,
                                     ones_f8, kcm, v_tm, te8, wq8, rq_b,
                                     mr_b, qs_v, qb_v, wo8, wo_b, te,
                                     x_dram, rec_dram, dram, fpre, f1t)

              if kph >= 6:
                  _ffn(nc, tc, psum, f1t, f2t, x_dram,
                       _emit.r2_d, _emit.mr2_d, f1s_v, f1_b,
                       f2_b, out, _emit.ffn_pre)


def _vision_kv(nc, tc, psum, vf8, vp8, wk8, wv8, vp_b, wk_b, wvb, kcm, v_tm):
    """pv = vp.T @ vf + vp_b (fp8); keys kcm = wk.T @ pv + wk_b (channel-major
    fp8); values v_tm = pv.T @ wv + wv_b (token-major fp8). All DoubleRow."""
    with tc.tile_pool(name="vision", bufs=1) as vision, \
         tc.tile_pool(name="vwork", bufs=3) as vwork:
        vf_sb = vision.tile([P, KO_DV, SVP], F8)
        nc.scalar.dma_start(vf_sb[:], vf8[:])
        wv_bb = vision.tile([P, DQP], F32)
        nc.gpsimd.dma_start(wv_bb[:], _pbcast(wvb[:]))
        pv = vision.tile([P, KO_DM, SVP], F8)
        for m in range(KO_DM):
            w_sl = vwork.tile([P, KO_DV, P], F8, tag="vp_sl")
            nc.scalar.dma_start(w_sl[:], vp8[m])
            ps = psum.tile([P, 512], F32, tag="ps_a")
            for k2 in range(KO_DV // 2):
                nc.tensor.matmul(ps[:, :SVP], w_sl[:, 2 * k2:2 * k2 + 2],
                                 vf_sb[:, 2 * k2:2 * k2 + 2],
                                 start=(k2 == 0), stop=(k2 == KO_DV // 2 - 1),
                                 perf_mode=DR)
            nc.scalar.activation(pv[:, m], ps[:, :SVP], AF.Identity,
                                 bias=vp_b[:, m:m + 1], scale=1.0 / WS)

        for m in range(KO_QP):
            w_sl = vwork.tile([P, KO_DM, P], F8, tag="wk_sl")
            nc.scalar.dma_start(w_sl[:], wk8[m])
            ps = psum.tile([P, 512], F32, tag="ps_a")
            for k2 in range(KO_DM // 2):
                nc.tensor.matmul(ps[:, :SVP], w_sl[:, 2 * k2:2 * k2 + 2],
                                 pv[:, 2 * k2:2 * k2 + 2],
                                 start=(k2 == 0), stop=(k2 == KO_DM // 2 - 1),
                                 perf_mode=DR)
            nc.scalar.activation(kcm[:, m], ps[:, :SVP], AF.Identity,
                                 bias=wk_b[:, m:m + 1], scale=1.0 / WS)

        for n in range(DQP // 512):
            w_sl = vwork.tile([P, KO_DM, 512], F8, tag="wv_sl", bufs=2)
            nc.scalar.dma_start(w_sl[:], wv8[n])
            for st in range(ST):
                ps = psum.tile([P, 512], F32, tag="ps_a")
                ssl = slice(st * P, (st + 1) * P)
                for k2 in range(KO_DM // 2):
                    nc.tensor.matmul(ps[:], pv[:, 2 * k2:2 * k2 + 2, ssl],
                                     w_sl[:, 2 * k2:2 * k2 + 2],
                                     start=(k2 == 0),
                                     stop=(k2 == KO_DM // 2 - 1),
                                     perf_mode=DR)
                nc.vector.scalar_tensor_tensor(
                    v_tm[:, st, n * 512:(n + 1) * 512], ps[:], 1.0 / WS,
                    wv_bb[:, n * 512:(n + 1) * 512], OP.mult, OP.add)


def _ln_stats(nc, tc, psums, pool, ones_bf, src_dram, dram, nm, rscale):
    """Per-token mean/rstd of a [P, KO_DM, SQ] bf16 DRAM tensor (reduction over
    channels=partitions via ones-matmuls).  Returns DRAM rows (rstd*rscale,
    -mean*rstd) ready for partition-broadcast."""
    src_sb = pool.tile([P, KO_DM, SQ], BF, tag=nm + "_src")
    for m in range(KO_DM):
        _dq(nc, m).dma_start(src_sb[:, m], src_dram[:, m])
    return _ln_stats_sb(nc, tc, psums, pool, ones_bf, src_sb, dram, nm, rscale)


def _ln_stats_sb(nc, tc, psums, pool, ones_bf, src_sb, dram, nm, rscale):
    """m-outer so each te chunk's stats issue as soon as its DMA lands."""
    with tc.tile_pool(name=nm + "w", bufs=3) as work:
        sums = pool.tile([1, SQ], F32, tag=nm + "_sums")
        sumsq = pool.tile([1, SQ], F32, tag=nm + "_sumsq")
        ps2s = [psums.tile([33, 512], F32, tag="ps_st", name=f"{nm}_ps{n}")
                for n in range(NT)]
        for m in range(KO_DM):
            sq = work.tile([P, SQ], BF, tag=nm + "_sq")
            nc.vector.tensor_mul(sq[:], src_sb[:, m], src_sb[:, m])
            for n in range(NT):
                nsl = slice(n * 512, (n + 1) * 512)
                nc.tensor.matmul(ps2s[n][0:1], ones_bf[:], src_sb[:, m, nsl],
                                 start=(m == 0), stop=(m == KO_DM - 1))
                nc.tensor.matmul(ps2s[n][32:33], ones_bf[:], sq[:, nsl],
                                 start=(m == 0), stop=(m == KO_DM - 1))
        for n in range(NT):
            nsl = slice(n * 512, (n + 1) * 512)
            nc.vector.tensor_copy(sums[:, nsl], ps2s[n][0:1])
            nc.vector.tensor_copy(sumsq[:, nsl], ps2s[n][32:33])
        return _ln_finalize(nc, pool, sums, sumsq, dram, nm, rscale)


def _ln_finalize(nc, pool, sums, sumsq, dram, nm, rscale, q=None):
    """-> DRAM rows (rstd*rscale [1,SQ], -mean*rstd [1,SQ])."""
    q = q if q is not None else nc.gpsimd
    tmp = pool.tile([1, SQ], F32, tag=nm + "_fin_tmp")
    nc.vector.tensor_scalar_mul(sums[:], sums[:], 1.0 / DM)      # mean
    nc.vector.tensor_scalar_mul(sumsq[:], sumsq[:], 1.0 / DM)
    nc.vector.scalar_tensor_tensor(tmp[:], sums[:], 1.0, sums[:],
                                   OP.mult, OP.mult)             # mean^2
    nc.vector.tensor_sub(sumsq[:], sumsq[:], tmp[:])             # var
    eps_t = pool.tile([1, 1], F32, tag=nm + "_eps")
    nc.vector.memset(eps_t[:], EPS)
    nc.scalar.activation(tmp[:], sumsq[:], AF.Sqrt, bias=eps_t[:])  # std
    nc.vector.reciprocal(sumsq[:], tmp[:])                       # rstd
    nc.vector.scalar_tensor_tensor(tmp[:], sums[:], -1.0, sumsq[:],
                                   OP.mult, OP.mult)             # -mean*rstd
    if rscale != 1.0:
        nc.vector.tensor_scalar_mul(sumsq[:], sumsq[:], rscale)
    # row bounces ride the SWDGE queue so they never block the HWDGE
    # weight/activation FIFOs
    r_d = dram.tile([1, SQ], F32, name=nm + "_r_dram")
    q.dma_start(r_d[:], sumsq[:])
    mr_d = dram.tile([1, SQ], F32, name=nm + "_mr_dram")
    q.dma_start(mr_d[:], tmp[:])
    return r_d, mr_d


def _q_attention(nc, tc, psum, psums, ones_bf, ones_f8, kcm, v_tm, te8, wq8,
                 rq_b, mr_b, qs_v, qb_v, wo8, wo_b, te, x_dram, rec_dram,
                 dram, fpre, f1t):
    """Q projection + blocked cross-attention + O projection + residual +
    LN2 stats, fully pipelined over 512-token blocks:
      Q(0) s(0) Q(1) s(1) c/O(0) Q(2) s(2) c/O(1) ... c/O(3)
    (scores run one block ahead of ctx/O to hide the reciprocal DRAM
    bounce; q never leaves SBUF).  x -> x_dram (bf16)."""
    with tc.tile_pool(name="attn", bufs=2) as attn, \
         tc.tile_pool(name="awork", bufs=2) as awork:
        sums2 = awork.tile([1, SQ], F32, tag="x2s", bufs=1, name="x2sums")
        sumsq2 = awork.tile([1, SQ], F32, tag="x2q", bufs=1, name="x2sumsq")

        def qproj(nb):
            nsl = slice(nb * NBS, (nb + 1) * NBS)
            te8b = attn.tile([P, KO_DM, NBS], F8, tag="te8b", bufs=1,
                             name=f"te8b{nb}")
            _dq(nc, nb).dma_start(te8b[:], te8[:, :, nsl])
            q_blk = attn.tile([P, KO_QP, NBS], F8, tag="q_blk",
                              name=f"q_blk{nb}")
            for m in range(KO_QP):
                w_sl = awork.tile([P, KO_DM, P], F8, tag="wq_sl", bufs=3)
                _dq(nc, m).dma_start(w_sl[:], wq8[m])
                ps = psum.tile([P, NBS], F32, tag="ps_a",
                               name=f"ps_q{nb}_{m}")
                for k2 in range(KO_DM // 2):
                    nc.tensor.matmul(ps[:], w_sl[:, 2 * k2:2 * k2 + 2],
                                     te8b[:, 2 * k2:2 * k2 + 2],
                                     start=(k2 == 0),
                                     stop=(k2 == KO_DM // 2 - 1),
                                     perf_mode=DR)
                tmp = awork.tile([P, NBS], F32, tag="q_tmp", bufs=3)
                nc.vector.tensor_mul(tmp[:], ps[:], rq_b[:, nsl])
                if qb_v is None:
                    nc.vector.scalar_tensor_tensor(
                        q_blk[:, m], mr_b[:, nsl], qs_v[:, m:m + 1],
                        tmp[:], OP.mult, OP.add)
                else:
                    nc.vector.scalar_tensor_tensor(
                        tmp[:], mr_b[:, nsl], qs_v[:, m:m + 1],
                        tmp[:], OP.mult, OP.add)
                    nc.vector.tensor_scalar_add(q_blk[:, m], tmp[:],
                                                qb_v[:, m:m + 1])
            return q_blk

        def scores(nb, q_blk):
            expT = attn.tile([P, H * ST, NBS], F8, tag="expT",
                             name=f"expT{nb}")
            rec = awork.tile([1, H * NBS], BF, tag="rec", bufs=1,
                             name=f"rec{nb}")
            for h in range(H):
                nc.vector.memset(expT[:, HC * h + HC - 1], 0.0)
                ps_sum = psums.tile([33, NBS], F32, tag="ps_st",
                                    name=f"ps_sum{nb}_{h}")
                for st in range(ST):
                    ps_s = psum.tile([P, NBS], F32, tag="ps_a",
                                     name=f"ps_s{nb}_{h}_{st}")
                    ssl = slice(st * P, (st + 1) * P)
                    nc.tensor.matmul(ps_s[:], kcm[:, HC * h:HC * h + 2, ssl],
                                     q_blk[:, HC * h:HC * h + 2],
                                     start=True, stop=False, perf_mode=DR)
                    nc.tensor.matmul(ps_s[:], kcm[:, HC * h + 2, ssl],
                                     q_blk[:, HC * h + 2],
                                     start=False, stop=True)
                    if st < ST - 1:
                        nc.scalar.activation(expT[:, HC * h + st], ps_s[:],
                                             AF.Exp, scale=SCALE)
                    else:
                        nc.scalar.activation(expT[0:1, HC * h + st],
                                             ps_s[0:1], AF.Exp, scale=SCALE)
                    nc.tensor.matmul(ps_sum[0:1], ones_f8[:],
                                     expT[:, HC * h + st],
                                     start=(st == 0), stop=(st == ST - 1))
                with nc.allow_low_precision(
                        reason="softmax recip in bf16 is plenty"):
                    nc.vector.reciprocal(rec[:, h * NBS:(h + 1) * NBS],
                                         ps_sum[0:1])
            roff = nb * H * NBS
            nc.gpsimd.dma_start(rec_dram[:, roff:roff + H * NBS], rec[:])
            rec_b = attn.tile([P, H * NBS], BF, tag="rec_b", bufs=1,
                              name=f"rec_b{nb}")
            nc.gpsimd.dma_start(rec_b[:],
                                _pbcast(rec_dram[:, roff:roff + H * NBS]))
            return expT, rec_b

        def ctx_o(nb, expT, rec_b):
            bsl = slice(nb * NBS, (nb + 1) * NBS)
            ctx8 = attn.tile([P, KO_QP, NBS], F8, tag="ctx8", bufs=1,
                             name=f"ctx8_{nb}")
            for h in range(H):
                for d3 in range(HC):
                    dsl = slice((HC * h + d3) * P, (HC * h + d3 + 1) * P)
                    ps_c = psum.tile([P, NBS], F32, tag="ps_a",
                                     name=f"ps_c{nb}_{h}_{d3}")
                    nc.tensor.matmul(ps_c[:], v_tm[:, 0:2, dsl],
                                     expT[:, HC * h:HC * h + 2],
                                     start=True, stop=False, perf_mode=DR)
                    nc.tensor.matmul(ps_c[:], v_tm[:, 2, dsl],
                                     expT[:, HC * h + 2],
                                     start=False, stop=True)
                    nc.vector.tensor_mul(ctx8[:, HC * h + d3], ps_c[:],
                                         rec_b[:, h * NBS:(h + 1) * NBS])

            # O projection + residual -> x_dram (bf16); LN2 stats inline.
            ps_st = psums.tile([33, NBS], F32, tag="ps_st",
                               name=f"ps_st{nb}")
            for m in range(KO_DM):
                w_sl = awork.tile([P, KO_QP, P], F8, tag="wo_sl", bufs=2)
                _dq(nc, m).dma_start(w_sl[:], wo8[m])
                te_sl = awork.tile([P, NBS], BF, tag="te_res")
                _dq(nc, m + 1).dma_start(te_sl[:], te[:, m, bsl])
                x_t = awork.tile([P, NBS], BF, tag="x_t")
                sq_t = awork.tile([P, NBS], BF, tag="sq_t")
                ps = psum.tile([P, NBS], F32, tag="ps_a",
                               name=f"ps_o{nb}_{m}")
                for k2 in range(KO_QP // 2):
                    nc.tensor.matmul(ps[:], w_sl[:, 2 * k2:2 * k2 + 2],
                                     ctx8[:, 2 * k2:2 * k2 + 2],
                                     start=(k2 == 0),
                                     stop=(k2 == KO_QP // 2 - 1),
                                     perf_mode=DR)
                if wo_b is None:
                    nc.vector.scalar_tensor_tensor(
                        x_t[:], ps[:], 1.0 / (WS * CR), te_sl[:],
                        OP.mult, OP.add)
                else:
                    tmp = awork.tile([P, NBS], F32, tag="xo_tmp")
                    nc.vector.scalar_tensor_tensor(
                        tmp[:], ps[:], 1.0 / (WS * CR), wo_b[:, m:m + 1],
                        OP.mult, OP.add)
                    nc.vector.tensor_add(x_t[:], tmp[:], te_sl[:])
                nc.tensor.matmul(ps_st[0:1], ones_bf[:], x_t[:],
                                 start=(m == 0), stop=(m == KO_DM - 1))
                nc.vector.tensor_mul(sq_t[:], x_t[:], x_t[:])
                nc.tensor.matmul(ps_st[32:33], ones_bf[:], sq_t[:],
                                 start=(m == 0), stop=(m == KO_DM - 1))
                _dq(nc, m).dma_start(x_dram[:, m, bsl], x_t[:])
            nc.vector.tensor_copy(sums2[:, bsl], ps_st[0:1])
            nc.vector.tensor_copy(sumsq2[:, bsl], ps_st[32:33])

        # scores run one block ahead of ctx/O to hide the reciprocal bounce
        pend = [scores(0, qproj(0))]
        for nb in range(NB):
            if nb + 1 < NB:
                pend.append(scores(nb + 1, qproj(nb + 1)))
            ctx_o(nb, *pend[nb])
            if nb == 0:
                # FFN prefetches live in fpre (reserved OUTSIDE the attention
                # pools, so no SBUF-reuse WAR) and start as soon as block 0's
                # x rows land in DRAM -- long before attention finishes.
                x0 = fpre.tile([P, KO_DM, 512], BF, name="x_blk0")
                nc.gpsimd.dma_start(x0[:], x_dram[:, :, 0:512])
                f1p = []
                for _m in range(2):
                    w = fpre.tile([P, KO_DM, P], BF, name=f"f1pre{_m}")
                    nc.gpsimd.dma_start(w[:], f1t[_m])
                    f1p.append(w)
                _emit.ffn_pre = (x0, f1p)
        _emit.r2_d, _emit.mr2_d = _ln_finalize(
            nc, awork, sums2, sumsq2, dram, "ln2", 1.0)


def _ffn(nc, tc, psum, f1t, f2t, x_dram, r2_d, mr2_d, f1s_v, f1_b, f2_b,
         out, ffn_pre):
    """Fused FFN over 512-token phases; h never leaves SBUF.
    FFN1(n): h = gelu(r2*(f1'.T @ x) + (-m2*r2)*colsum(f1') + f1_b')
    FFN2(n): out = f2.T @ h + f2_b + x   (residual on device)
    f1 streams on the sync queue, f2 on the scalar queue; both are re-read
    per phase (~170 MB each over the whole FFN span -- well under the HBM
    budget and fully overlapped)."""
    with tc.tile_pool(name="hpool", bufs=1) as hpool, \
         tc.tile_pool(name="fwork", bufs=2) as fwork:
        x0_pre, pre = ffn_pre
        for n in range(NT):
            nsl = slice(n * 512, (n + 1) * 512)
            if n == 0:
                x_blk = x0_pre
            else:
                x_blk = fwork.tile([P, KO_DM, 512], BF, tag="x_blk",
                                   name=f"x_blk{n}")
                nc.gpsimd.dma_start(x_blk[:], x_dram[:, :, nsl])
            r2s = fwork.tile([P, 512], F32, tag="r2s", name=f"r2s{n}")
            nc.gpsimd.dma_start(r2s[:], _pbcast(r2_d[:, nsl]))
            mr2s = fwork.tile([P, 512], F32, tag="mr2s", name=f"mr2s{n}")
            nc.gpsimd.dma_start(mr2s[:], _pbcast(mr2_d[:, nsl]))
            h_sb = hpool.tile([P, KO_DF, 512], BF, tag="h_sb",
                              name=f"h_sb{n}")
            for m in range(KO_DF):
                if n == 0 and m < len(pre):
                    w_sl = pre[m]
                else:
                    w_sl = fwork.tile([P, KO_DM, P], BF, tag="f1_sl", bufs=3)
                    nc.sync.dma_start(w_sl[:], f1t[m])
                ps = psum.tile([P, 512], F32, tag="ps_a",
                               name=f"ps_f1_{n}_{m}")
                for k in range(KO_DM):
                    nc.tensor.matmul(ps[:], w_sl[:, k], x_blk[:, k],
                                     start=(k == 0), stop=(k == KO_DM - 1))
                tmp = fwork.tile([P, 512], F32, tag="h_tmp", bufs=3)
                nc.vector.tensor_mul(tmp[:], ps[:], r2s[:])
                nc.vector.scalar_tensor_tensor(tmp[:], mr2s[:],
                                               f1s_v[:, m:m + 1], tmp[:],
                                               OP.mult, OP.add)
                nc.scalar.activation(h_sb[:, m], tmp[:], AF.Gelu,
                                     bias=f1_b[:, m:m + 1])
            for m in range(KO_DM):
                w2a = fwork.tile([P, KO_DF // 2, P], BF, tag="f2_sl",
                                 bufs=3, name=f"w2a{n}_{m}")
                nc.scalar.dma_start(w2a[:], f2t[m][:, 0:KO_DF // 2])
                w2b = fwork.tile([P, KO_DF // 2, P], BF, tag="f2_sl",
                                 bufs=3, name=f"w2b{n}_{m}")
                nc.scalar.dma_start(w2b[:], f2t[m][:, KO_DF // 2:KO_DF])
                ps = psum.tile([P, 512], F32, tag="ps_a",
                               name=f"ps_f2_{n}_{m}")
                for k in range(KO_DF):
                    w2 = w2a if k < KO_DF // 2 else w2b
                    nc.tensor.matmul(ps[:], w2[:, k % (KO_DF // 2)],
                                     h_sb[:, k],
                                     start=(k == 0), stop=(k == KO_DF - 1))
                o_sl = fwork.tile([P, 512], F32, tag="o_sl")
                nc.vector.scalar_tensor_tensor(
                    o_sl[:], ps[:], f2_b[:, m:m + 1], x_blk[:, m],
                    OP.add, OP.add)
                _dq(nc, m).dma_start(out[:, m, nsl], o_sl[:])


# ------------------------------------------------------------- host wrappers

def _tile_w(w, ko, mo, dtype):
    """[K, M] weight -> [mo, 128, ko, mi] SBUF-image tiles."""
    K, M = w.shape
    mi = M // mo
    r = w.reshape(ko, P, mo, mi).transpose(2, 1, 0, 3)
    return np.ascontiguousarray(r.astype(dtype))


def _col_pad_heads(w):
    """[*, 2304] -> [*, 3072] zero-padding each head's 288 cols to 384."""
    r = np.zeros(w.shape[:-1] + (DQP,), np.float32)
    r.reshape(w.shape[:-1] + (H, DKP))[..., :DK] = \
        w.reshape(w.shape[:-1] + (H, DK))
    return r


def _row_pad_heads(w):
    """[2304, *] -> [3072, *] zero-padding each head's 288 rows to 384."""
    r = np.zeros((DQP,) + w.shape[1:], np.float32)
    r.reshape((H, DKP) + w.shape[1:])[:, :DK] = w.reshape((H, DK) + w.shape[1:])
    return r


def _vec_t(v, ko):
    """[ko*128] vector -> [128, ko] f32."""
    return np.ascontiguousarray(v.reshape(ko, P).T.astype(np.float32))


def _make_in_maps(inputs):
    inputs = {k: np.asarray(v, np.float32) for k, v in inputs.items()}

    ln1w = inputs["ln1_w"][:, None]
    ln2w = inputs["ln2_w"][:, None]
    wq_f = _col_pad_heads(inputs["wq_w"] * ln1w)        # ln1_w folded
    wk_pad = _col_pad_heads(inputs["wk_w"])
    wv_pad = _col_pad_heads(inputs["wv_w"])
    wo_pad = _row_pad_heads(inputs["wo_w"])
    f1_f = inputs["f1_w"] * ln2w                        # ln2_w folded

    qb = _col_pad_heads((inputs["wq_b"]
                         + inputs["ln1_b"] @ inputs["wq_w"])[None])[0]
    f1b = inputs["f1_b"] + inputs["ln2_b"] @ inputs["f1_w"]
    has_qb = bool(np.any(qb))
    has_wob = bool(np.any(inputs["wo_b"]))

    shared = {
        "vp8": _tile_w(inputs["vp_w"] * WS, KO_DV, KO_DM, f8e4),
        "wq8": _tile_w(wq_f * WS, KO_DM, KO_QP, f8e4),
        "wk8": _tile_w(wk_pad * WS, KO_DM, KO_QP, f8e4),
        "wv8": _tile_w(wv_pad * WS, KO_DM, DQP // 512, f8e4),
        "wo8": _tile_w(wo_pad * WS, KO_QP, KO_DM, f8e4),
        "f1t": _tile_w(f1_f, KO_DM, KO_DF, bf16),
        "f2t": _tile_w(inputs["f2_w"], KO_DF, KO_DM, bf16),
        "vp_bt": _vec_t(inputs["vp_b"], KO_DM),
        "wkb_t": _vec_t(_col_pad_heads(inputs["wk_b"][None])[0], KO_QP),
        "qs_c": _vec_t(wq_f.sum(axis=0), KO_QP),
        "qb_c": _vec_t(qb, KO_QP),
        "wvb": np.ascontiguousarray(
            _col_pad_heads(inputs["wv_b"][None]).astype(np.float32)),
        "wob_t": _vec_t(inputs["wo_b"], KO_DM),
        "f1b_t": _vec_t(f1b, KO_DF),
        "f1s_c": _vec_t(f1_f.sum(axis=0), KO_DF),
        "f2b_t": _vec_t(inputs["f2_b"], KO_DM),
    }

    text = inputs["text_embeddings"]
    vision = inputs["vision_features"]
    in_maps = []
    for b in range(B):
        te_cm = np.ascontiguousarray(
            text[b].T.reshape(KO_DM, P, SQ).transpose(1, 0, 2))
        vf_pad = np.zeros((DV, SVP), np.float32)
        vf_pad[:, :SV] = vision[b].T
        vf_b = np.ascontiguousarray(
            vf_pad.reshape(KO_DV, P, SVP).transpose(1, 0, 2).astype(f8e4))
        in_maps.append({"te": te_cm.astype(bf16), "te8": te_cm.astype(f8e4),
                        "vf8": vf_b, **shared})
    return in_maps, has_qb, has_wob


def kernel(**inputs):
    in_maps, has_qb, has_wob = _make_in_maps(inputs)

    key = ("nc", has_qb, has_wob)
    if key not in _NC_CACHE:
        _NC_CACHE[key] = _build_nc(has_qb, has_wob)
        _NC_CACHE["nc"] = _NC_CACHE[key]
    nc = _NC_CACHE[key]

    res = run_bass_kernel_spmd(nc, in_maps, core_ids=list(range(B)))

    outs = []
    for b in range(B):
        r = res.results[b]["out"]                       # [128, 18, 2048]
        outs.append(r.transpose(1, 0, 2).reshape(DM, SQ).T)
    return np.stack(outs).astype(np.float32)


if __name__ == "__main__":
    import reference
    inp = {k: np.asarray(v) for k, v in reference.setup_inputs().items()}
    got = kernel(**inp)
    exp = np.asarray(reference.reference(**inp))
    err = float(np.linalg.norm(got - exp) / np.linalg.norm(exp))
    print("Relative error:", err)
